# revision 5
# baseline (speedup 1.0000x reference)
"""Trainium2 Bass kernel for nn_D_loss_67551245631962.

Computes: 0.8 * sum(WMA5(target_angle - pred_angle)^2) + 0.2 * sum((target_class - pred_class)^2)
where WMA5 is a 5-tap [0.05, 0.1, 0.7, 0.1, 0.05] correlation with 2-zero padding per side.

Strategy (pure data parallelism over batch dim B=2048 across 8 cores, 256 rows/core):
  - SWDGE (gpsimd) DMA loads target_angle with fp32->fp16 cast, then pred_angle with
    cast + accum_op=subtract: the DMA itself materializes diff = pred - target in SBUF
    (sign irrelevant, result is squared). Zero compute-engine passes for the subtract.
  - ACT (scalar engine) makes a 1-element-shifted fp16 copy so every DVE operand is
    4-byte aligned (required for the DVE 2x_1P perf mode on 16-bit dtypes).
  - DVE runs the 5-tap conv as 4 fused scalar_tensor_tensor ops (Horner form with
    weight ratios; final *w4 folded into the host-side scale).
  - ACT does fused Square + accum_out reduction -> per-partition partial sums.
  - Host sums 8 cores' [128, NACC] partials in float64 and applies 0.8*w4^2 / 0.2.

Per-core engine budget (target memory-bound):  DMA ~47us (16.8 MB @ ~358 GB/s),
DVE 4 STT @ 2x ~34us, ACT (shift-copy + square-accum) ~30us.
"""

import os
import sys

for _p in ("/opt/trn_rl_repo",):
    if os.path.isdir(_p) and _p not in sys.path:
        sys.path.insert(0, _p)

from contextlib import ExitStack

import numpy as np

import concourse.bass as bass
import concourse.tile as tile
from concourse import bacc, mybir
from concourse.bass_utils import run_bass_kernel_spmd

N_CORES = 8
B, T = 2048, 8192
RPC = B // N_CORES  # rows per core = 256
G = RPC // 128      # 128-partition row groups per core = 2
F = 2048            # free-dim tile size (conv output cols per tile)
NT = T // F         # column tiles per group
NACC = G * NT + G   # accumulator columns: G*NT angle + G class

W = (0.05, 0.1, 0.7, 0.1, 0.05)
# Horner ratios: wma = ((((d0*r1 + d1)*r2 + d2)*r3 + d3)*r4 + d4) * W[4]
R1 = W[0] / W[1]
R2 = W[1] / W[2]
R3 = W[2] / W[3]
R4 = W[3] / W[4]

DT16 = mybir.dt.float16  # conv compute storage dtype (2-byte => DVE 2x mode)


def build_nc():
    nc = bacc.Bacc("TRN2")
    dt = mybir.dt
    ta = nc.dram_tensor("target_angle", [RPC, T], dt.float32, kind="ExternalInput")
    pa = nc.dram_tensor("pred_angle", [RPC, T], dt.float32, kind="ExternalInput")
    tcl = nc.dram_tensor("target_class", [RPC, 3], dt.float32, kind="ExternalInput")
    pcl = nc.dram_tensor("pred_class", [RPC, 3], dt.float32, kind="ExternalInput")
    out = nc.dram_tensor("out", [128, NACC], dt.float32, kind="ExternalOutput")

    AF = mybir.ActivationFunctionType
    OP = mybir.AluOpType

    with tile.TileContext(nc) as tc, ExitStack() as ctx:
        ppool = ctx.enter_context(tc.tile_pool(name="paf", bufs=3))
        dpool = ctx.enter_context(tc.tile_pool(name="dbf", bufs=4))
        hpool = ctx.enter_context(tc.tile_pool(name="dbs", bufs=3))
        spool = ctx.enter_context(tc.tile_pool(name="s", bufs=4))
        jpool = ctx.enter_context(tc.tile_pool(name="junk", bufs=2))
        apool = ctx.enter_context(tc.tile_pool(name="acc", bufs=1))
        cpool = ctx.enter_context(tc.tile_pool(name="cls", bufs=2))

        accums = apool.tile([128, NACC], dt.float32)

        for g in range(G):
            r0, r1_ = g * 128, (g + 1) * 128
            for t in range(NT):
                # tile covers diff cols [t*F-2, t*F+F+2) (2-col halo each side)
                lo, hi = t * F - 2, t * F + F + 2
                dst_lo, dst_hi = 0, F + 4
                if lo < 0:
                    dst_lo, lo = 2, 0
                if hi > T:
                    dst_hi, hi = F + 2, T

                # fp32 pred_angle via HWDGE; pad halo cols zeroed
                paf = ppool.tile([128, F + 4], dt.float32, tag="paf")
                if dst_lo:
                    nc.vector.memset(paf[:, 0:dst_lo], 0.0)
                if dst_hi < F + 4:
                    nc.vector.memset(paf[:, dst_hi : F + 4], 0.0)
                nc.sync.dma_start(paf[:, dst_lo:dst_hi], pa[r0:r1_, lo:hi])

                # ACT: negate + cast fp32 -> fp16  (dbf = -pred)
                dbf = dpool.tile([128, F + 4], DT16, tag="dbf")
                nc.scalar.activation(dbf[:], paf[:], AF.Copy, scale=-1.0)

                # SWDGE: cast + accumulate-add target on top (dbf = target - pred)
                nc.gpsimd.dma_start(
                    dbf[:, dst_lo:dst_hi], ta[r0:r1_, lo:hi], accum_op=OP.add
                )

                # 1-element shifted copy on ACT so odd-tap operands are 4B-aligned
                dbs = hpool.tile([128, F + 2], DT16, tag="dbs")
                nc.scalar.copy(dbs[:], dbf[:, 1 : F + 3])

                # 5-tap conv, Horner form, 4 fused STT ops on DVE (all 2x-aligned)
                s1 = spool.tile([128, F], DT16, tag="s")
                nc.vector.scalar_tensor_tensor(
                    s1[:], dbf[:, 0:F], R1, dbs[:, 0:F], op0=OP.mult, op1=OP.add
                )
                s2 = spool.tile([128, F], DT16, tag="s")
                nc.vector.scalar_tensor_tensor(
                    s2[:], s1[:], R2, dbf[:, 2 : F + 2], op0=OP.mult, op1=OP.add
                )
                s3 = spool.tile([128, F], DT16, tag="s")
                nc.vector.scalar_tensor_tensor(
                    s3[:], s2[:], R3, dbs[:, 2 : F + 2], op0=OP.mult, op1=OP.add
                )
                s4 = spool.tile([128, F], DT16, tag="s")
                nc.vector.scalar_tensor_tensor(
                    s4[:], s3[:], R4, dbf[:, 4 : F + 4], op0=OP.mult, op1=OP.add
                )

                # fused square + per-partition reduction on ACT
                junk = jpool.tile([128, F], DT16, tag="junk")
                col = g * NT + t
                nc.scalar.activation(
                    junk[:], s4[:], AF.Square, accum_out=accums[:, col : col + 1]
                )

            # class SSE for this row group (tiny)
            ct = cpool.tile([128, 3], dt.float32, tag="cls")
            cp = cpool.tile([128, 3], dt.float32, tag="clsp")
            nc.sync.dma_start(ct[:], tcl[r0:r1_, :])
            nc.sync.dma_start(cp[:], pcl[r0:r1_, :])
            cd = cpool.tile([128, 3], dt.float32, tag="clsd")
            nc.vector.tensor_sub(cd[:], ct[:], cp[:])
            cj = cpool.tile([128, 3], dt.float32, tag="clsj")
            col = G * NT + g
            nc.scalar.activation(
                cj[:], cd[:], AF.Square, accum_out=accums[:, col : col + 1]
            )

        nc.sync.dma_start(out[:], accums[:])

    nc.finalize()
    return nc


_NC = None
last_result = None  # BassKernelResults of the most recent run (for test harness)


def kernel(target_angle, pred_angle, target_class, pred_class):
    global _NC, last_result
    if _NC is None:
        _NC = build_nc()

    in_maps = []
    for c in range(N_CORES):
        r = slice(c * RPC, (c + 1) * RPC)
        in_maps.append(
            {
                "target_angle": np.ascontiguousarray(target_angle[r], dtype=np.float32),
                "pred_angle": np.ascontiguousarray(pred_angle[r], dtype=np.float32),
                "target_class": np.ascontiguousarray(target_class[r], dtype=np.float32),
                "pred_class": np.ascontiguousarray(pred_class[r], dtype=np.float32),
            }
        )

    last_result = run_bass_kernel_spmd(
        _NC,
        in_maps,
        core_ids=list(range(N_CORES)),
        trace=bool(os.environ.get("BASS_TRACE")),
    )

    angle = 0.0
    cls = 0.0
    for r in last_result.results:
        o = np.asarray(r["out"], dtype=np.float64)
        angle += o[:, 0 : G * NT].sum()
        cls += o[:, G * NT : NACC].sum()

    val = 0.8 * (W[4] * W[4]) * angle + 0.2 * cls
    return np.array(val, dtype=np.float32)


# revision 8
# speedup vs baseline: 1.2220x; 1.2220x over previous
"""Trainium2 Bass kernel for nn_D_loss_67551245631962.

Computes: 0.8 * sum(WMA5(target_angle - pred_angle)^2) + 0.2 * sum((target_class - pred_class)^2)
where WMA5 is a 5-tap [0.05, 0.1, 0.7, 0.1, 0.05] correlation with 2-zero padding per side.

Strategy (pure data parallelism over batch dim B=2048 across 8 cores, 256 rows/core):
  - HWDGE DMA loads pred_angle fp32; ACT negates+casts to fp16 (activation Copy,
    scale=-1); SWDGE (gpsimd) DMA then cast+accum_op=add's target_angle on top:
    the DMA materializes diff = target - pred in fp16 with no DVE passes.
    (CCE only supports add, not subtract - hence the negate-then-add.)
  - DVE computes the symmetric 5-tap conv as s = 14*d2 + 2*(d1+d3) + (d0+d4)
    using 4 tensor_tensor adds (2x mode) + 2 tensor_scalar muls (4x mode);
    wma = 0.05*s with the 0.05 folded into the host-side scale. HW-verified:
    fp16 TT=2x even at odd-element offsets; scalar_tensor_tensor is 1x (avoid).
  - ACT does fused Square + accum_out reduction -> per-partition partial sums.
  - Host sums 8 cores' [128, NACC] partials in float64 and applies 0.8*0.05^2 / 0.2.

Per-core engine budget (target memory-bound): DMA ~47us (16.8 MB @ ~358 GB/s),
DVE 2.5f ~46us, ACT (negcast + square-accum) ~33us.
"""

import os
import sys

for _p in ("/opt/trn_rl_repo",):
    if os.path.isdir(_p) and _p not in sys.path:
        sys.path.insert(0, _p)

from contextlib import ExitStack

import numpy as np

import concourse.bass as bass
import concourse.tile as tile
from concourse import bacc, mybir
from concourse.bass_utils import run_bass_kernel_spmd

N_CORES = 8
B, T = 2048, 8192
RPC = B // N_CORES  # rows per core = 256
G = RPC // 128      # 128-partition row groups per core = 2
F = 2048            # free-dim tile size (conv output cols per tile)
NT = T // F         # column tiles per group
NACC = G * NT + G   # accumulator columns: G*NT angle + G class

W = (0.05, 0.1, 0.7, 0.1, 0.05)

DT16 = mybir.dt.float16  # conv compute storage dtype (2-byte => DVE 2x mode)


def build_nc():
    nc = bacc.Bacc("TRN2")
    dt = mybir.dt
    ta = nc.dram_tensor("target_angle", [RPC, T], dt.float32, kind="ExternalInput")
    pa = nc.dram_tensor("pred_angle", [RPC, T], dt.float32, kind="ExternalInput")
    tcl = nc.dram_tensor("target_class", [RPC, 3], dt.float32, kind="ExternalInput")
    pcl = nc.dram_tensor("pred_class", [RPC, 3], dt.float32, kind="ExternalInput")
    out = nc.dram_tensor("out", [128, NACC], dt.float32, kind="ExternalOutput")

    AF = mybir.ActivationFunctionType
    OP = mybir.AluOpType

    with tile.TileContext(nc) as tc, ExitStack() as ctx:
        ppool = ctx.enter_context(tc.tile_pool(name="paf", bufs=3))
        dpool = ctx.enter_context(tc.tile_pool(name="dbf", bufs=4))
        spool = ctx.enter_context(tc.tile_pool(name="s", bufs=6))
        jpool = ctx.enter_context(tc.tile_pool(name="junk", bufs=2))
        apool = ctx.enter_context(tc.tile_pool(name="acc", bufs=1))
        cpool = ctx.enter_context(tc.tile_pool(name="cls", bufs=2))

        accums = apool.tile([128, NACC], dt.float32)

        for g in range(G):
            r0, r1_ = g * 128, (g + 1) * 128
            for t in range(NT):
                # tile covers diff cols [t*F-2, t*F+F+2) (2-col halo each side)
                lo, hi = t * F - 2, t * F + F + 2
                dst_lo, dst_hi = 0, F + 4
                if lo < 0:
                    dst_lo, lo = 2, 0
                if hi > T:
                    dst_hi, hi = F + 2, T

                # fp32 pred_angle via HWDGE; pad halo cols zeroed
                paf = ppool.tile([128, F + 4], dt.float32, tag="paf")
                if dst_lo:
                    nc.vector.memset(paf[:, 0:dst_lo], 0.0)
                if dst_hi < F + 4:
                    nc.vector.memset(paf[:, dst_hi : F + 4], 0.0)
                nc.sync.dma_start(paf[:, dst_lo:dst_hi], pa[r0:r1_, lo:hi])

                # ACT: negate + cast fp32 -> fp16  (dbf = -pred)
                dbf = dpool.tile([128, F + 4], DT16, tag="dbf")
                nc.scalar.activation(dbf[:], paf[:], AF.Copy, scale=-1.0)

                # SWDGE: cast + accumulate-add target on top (dbf = target - pred)
                nc.gpsimd.dma_start(
                    dbf[:, dst_lo:dst_hi], ta[r0:r1_, lo:hi], accum_op=OP.add
                )

                # 5-tap symmetric conv on DVE: s = 14*d2 + 2*(d1+d3) + (d0+d4),
                # wma = 0.05*s (0.05 folded into host scale). TT=2x, TS=4x on fp16;
                # odd-offset slices still hit 2x (HW-verified).
                u = spool.tile([128, F], DT16, tag="s")
                nc.vector.tensor_add(u[:], dbf[:, 1 : F + 1], dbf[:, 3 : F + 3])
                p = spool.tile([128, F], DT16, tag="s")
                nc.vector.tensor_scalar_mul(p[:], dbf[:, 2 : F + 2], 7.0)
                v = spool.tile([128, F], DT16, tag="s")
                nc.vector.tensor_add(v[:], dbf[:, 0:F], dbf[:, 4 : F + 4])
                x = spool.tile([128, F], DT16, tag="s")
                nc.vector.tensor_add(x[:], p[:], u[:])
                y = spool.tile([128, F], DT16, tag="s")
                nc.vector.tensor_scalar_mul(y[:], x[:], 2.0)
                s4 = spool.tile([128, F], DT16, tag="s")
                nc.vector.tensor_add(s4[:], y[:], v[:])

                # fused square + per-partition reduction on ACT
                junk = jpool.tile([128, F], DT16, tag="junk")
                col = g * NT + t
                nc.scalar.activation(
                    junk[:], s4[:], AF.Square, accum_out=accums[:, col : col + 1]
                )

            # class SSE for this row group (tiny)
            ct = cpool.tile([128, 3], dt.float32, tag="cls")
            cp = cpool.tile([128, 3], dt.float32, tag="clsp")
            nc.sync.dma_start(ct[:], tcl[r0:r1_, :])
            nc.sync.dma_start(cp[:], pcl[r0:r1_, :])
            cd = cpool.tile([128, 3], dt.float32, tag="clsd")
            nc.vector.tensor_sub(cd[:], ct[:], cp[:])
            cj = cpool.tile([128, 3], dt.float32, tag="clsj")
            col = G * NT + g
            nc.scalar.activation(
                cj[:], cd[:], AF.Square, accum_out=accums[:, col : col + 1]
            )

        nc.sync.dma_start(out[:], accums[:])

    nc.finalize()
    return nc


_NC = None
last_result = None  # BassKernelResults of the most recent run (for test harness)


def kernel(target_angle, pred_angle, target_class, pred_class):
    global _NC, last_result
    if _NC is None:
        _NC = build_nc()

    in_maps = []
    for c in range(N_CORES):
        r = slice(c * RPC, (c + 1) * RPC)
        in_maps.append(
            {
                "target_angle": np.ascontiguousarray(target_angle[r], dtype=np.float32),
                "pred_angle": np.ascontiguousarray(pred_angle[r], dtype=np.float32),
                "target_class": np.ascontiguousarray(target_class[r], dtype=np.float32),
                "pred_class": np.ascontiguousarray(pred_class[r], dtype=np.float32),
            }
        )

    last_result = run_bass_kernel_spmd(
        _NC,
        in_maps,
        core_ids=list(range(N_CORES)),
        trace=bool(os.environ.get("BASS_TRACE")),
    )

    angle = 0.0
    cls = 0.0
    for r in last_result.results:
        o = np.asarray(r["out"], dtype=np.float64)
        angle += o[:, 0 : G * NT].sum()
        cls += o[:, G * NT : NACC].sum()

    val = 0.8 * (W[4] * W[4]) * angle + 0.2 * cls
    return np.array(val, dtype=np.float32)


# revision 9
# speedup vs baseline: 1.2283x; 1.0051x over previous
"""Trainium2 Bass kernel for nn_D_loss_67551245631962.

Computes: 0.8 * sum(WMA5(target_angle - pred_angle)^2) + 0.2 * sum((target_class - pred_class)^2)
where WMA5 is a 5-tap [0.05, 0.1, 0.7, 0.1, 0.05] correlation with 2-zero padding per side.

Strategy (pure data parallelism over batch dim B=2048 across 8 cores, 256 rows/core):
  - HWDGE DMA loads pred_angle fp32; ACT negates+casts to fp16 (activation Copy,
    scale=-1); SWDGE (gpsimd) DMA then cast+accum_op=add's target_angle on top:
    the DMA materializes diff = target - pred in fp16 with no DVE passes.
    (CCE only supports add, not subtract - hence the negate-then-add.)
  - DVE computes the symmetric 5-tap conv as s = 14*d2 + 2*(d1+d3) + (d0+d4)
    using 4 tensor_tensor adds (2x mode) + 2 tensor_scalar muls (4x mode);
    wma = 0.05*s with the 0.05 folded into the host-side scale. HW-verified:
    fp16 TT=2x even at odd-element offsets; scalar_tensor_tensor is 1x (avoid).
  - ACT does fused Square + accum_out reduction -> per-partition partial sums.
  - Host sums 8 cores' [128, NACC] partials in float64 and applies 0.8*0.05^2 / 0.2.

Per-core engine budget (target memory-bound): DMA ~47us (16.8 MB @ ~358 GB/s),
DVE 2.5f ~46us, ACT (negcast + square-accum) ~33us.
"""

import os
import sys

for _p in ("/opt/trn_rl_repo",):
    if os.path.isdir(_p) and _p not in sys.path:
        sys.path.insert(0, _p)

from contextlib import ExitStack

import numpy as np

import concourse.bass as bass
import concourse.tile as tile
from concourse import bacc, mybir
from concourse.bass_utils import run_bass_kernel_spmd

N_CORES = 8
B, T = 2048, 8192
RPC = B // N_CORES  # rows per core = 256
G = RPC // 128      # 128-partition row groups per core = 2
F = 2048            # free-dim tile size (conv output cols per tile)
NT = T // F         # column tiles per group
NACC = G * NT + G   # accumulator columns: G*NT angle + G class

W = (0.05, 0.1, 0.7, 0.1, 0.05)

DT16 = mybir.dt.float16  # conv compute storage dtype (2-byte => DVE 2x mode)


def build_nc():
    nc = bacc.Bacc("TRN2")
    dt = mybir.dt
    ta = nc.dram_tensor("target_angle", [RPC, T], dt.float32, kind="ExternalInput")
    pa = nc.dram_tensor("pred_angle", [RPC, T], dt.float32, kind="ExternalInput")
    tcl = nc.dram_tensor("target_class", [RPC, 3], dt.float32, kind="ExternalInput")
    pcl = nc.dram_tensor("pred_class", [RPC, 3], dt.float32, kind="ExternalInput")
    out = nc.dram_tensor("out", [128, NACC], dt.float32, kind="ExternalOutput")

    AF = mybir.ActivationFunctionType
    OP = mybir.AluOpType

    with tile.TileContext(nc) as tc, ExitStack() as ctx:
        ppool = ctx.enter_context(tc.tile_pool(name="paf", bufs=4))
        dpool = ctx.enter_context(tc.tile_pool(name="dbf", bufs=G * NT))
        spool = ctx.enter_context(tc.tile_pool(name="s", bufs=6))
        jpool = ctx.enter_context(tc.tile_pool(name="junk", bufs=2))
        apool = ctx.enter_context(tc.tile_pool(name="acc", bufs=1))
        cpool = ctx.enter_context(tc.tile_pool(name="cls", bufs=2))

        accums = apool.tile([128, NACC], dt.float32)

        # Phase A (emitted first so ACT's in-order stream does every negcast
        # before any square -- otherwise square(t) gates negcast(t+1) and the
        # tiles serialize): load pred, negcast, CCE-add target. All 8 dbf
        # tiles live simultaneously (dpool bufs = G*NT).
        dbfs = []
        for g in range(G):
            r0, r1_ = g * 128, (g + 1) * 128
            for t in range(NT):
                # tile covers diff cols [t*F-2, t*F+F+2) (2-col halo each side)
                lo, hi = t * F - 2, t * F + F + 2
                dst_lo, dst_hi = 0, F + 4
                if lo < 0:
                    dst_lo, lo = 2, 0
                if hi > T:
                    dst_hi, hi = F + 2, T

                # fp32 pred_angle via HWDGE; pad halo cols zeroed
                paf = ppool.tile([128, F + 4], dt.float32, tag="paf")
                if dst_lo:
                    nc.vector.memset(paf[:, 0:dst_lo], 0.0)
                if dst_hi < F + 4:
                    nc.vector.memset(paf[:, dst_hi : F + 4], 0.0)
                nc.sync.dma_start(paf[:, dst_lo:dst_hi], pa[r0:r1_, lo:hi])

                # ACT: negate + cast fp32 -> fp16  (dbf = -pred)
                dbf = dpool.tile([128, F + 4], DT16, tag="dbf")
                nc.scalar.activation(dbf[:], paf[:], AF.Copy, scale=-1.0)

                # SWDGE: cast + accumulate-add target on top (dbf = target - pred)
                nc.gpsimd.dma_start(
                    dbf[:, dst_lo:dst_hi], ta[r0:r1_, lo:hi], accum_op=OP.add
                )
                dbfs.append(dbf)

        # Phase B: per tile, 5-tap symmetric conv on DVE
        # s = 14*d2 + 2*(d1+d3) + (d0+d4); wma = 0.05*s (0.05 folded into host
        # scale). TT=2x, TS=4x on fp16; odd-offset slices still hit 2x
        # (HW-verified). Then fused Square + accum on ACT.
        for i, dbf in enumerate(dbfs):
            u = spool.tile([128, F], DT16, tag="s")
            nc.vector.tensor_add(u[:], dbf[:, 1 : F + 1], dbf[:, 3 : F + 3])
            p = spool.tile([128, F], DT16, tag="s")
            nc.vector.tensor_scalar_mul(p[:], dbf[:, 2 : F + 2], 7.0)
            v = spool.tile([128, F], DT16, tag="s")
            nc.vector.tensor_add(v[:], dbf[:, 0:F], dbf[:, 4 : F + 4])
            x = spool.tile([128, F], DT16, tag="s")
            nc.vector.tensor_add(x[:], p[:], u[:])
            y = spool.tile([128, F], DT16, tag="s")
            nc.vector.tensor_scalar_mul(y[:], x[:], 2.0)
            s4 = spool.tile([128, F], DT16, tag="s")
            nc.vector.tensor_add(s4[:], y[:], v[:])

            junk = jpool.tile([128, F], DT16, tag="junk")
            nc.scalar.activation(
                junk[:], s4[:], AF.Square, accum_out=accums[:, i : i + 1]
            )

        # class SSE per row group (tiny)
        for g in range(G):
            r0, r1_ = g * 128, (g + 1) * 128
            ct = cpool.tile([128, 3], dt.float32, tag="cls")
            cp = cpool.tile([128, 3], dt.float32, tag="clsp")
            nc.sync.dma_start(ct[:], tcl[r0:r1_, :])
            nc.sync.dma_start(cp[:], pcl[r0:r1_, :])
            cd = cpool.tile([128, 3], dt.float32, tag="clsd")
            nc.vector.tensor_sub(cd[:], ct[:], cp[:])
            cj = cpool.tile([128, 3], dt.float32, tag="clsj")
            col = G * NT + g
            nc.scalar.activation(
                cj[:], cd[:], AF.Square, accum_out=accums[:, col : col + 1]
            )

        nc.sync.dma_start(out[:], accums[:])

    nc.finalize()
    return nc


_NC = None
last_result = None  # BassKernelResults of the most recent run (for test harness)


def kernel(target_angle, pred_angle, target_class, pred_class):
    global _NC, last_result
    if _NC is None:
        _NC = build_nc()

    in_maps = []
    for c in range(N_CORES):
        r = slice(c * RPC, (c + 1) * RPC)
        in_maps.append(
            {
                "target_angle": np.ascontiguousarray(target_angle[r], dtype=np.float32),
                "pred_angle": np.ascontiguousarray(pred_angle[r], dtype=np.float32),
                "target_class": np.ascontiguousarray(target_class[r], dtype=np.float32),
                "pred_class": np.ascontiguousarray(pred_class[r], dtype=np.float32),
            }
        )

    last_result = run_bass_kernel_spmd(
        _NC,
        in_maps,
        core_ids=list(range(N_CORES)),
        trace=bool(os.environ.get("BASS_TRACE")),
    )

    angle = 0.0
    cls = 0.0
    for r in last_result.results:
        o = np.asarray(r["out"], dtype=np.float64)
        angle += o[:, 0 : G * NT].sum()
        cls += o[:, G * NT : NACC].sum()

    val = 0.8 * (W[4] * W[4]) * angle + 0.2 * cls
    return np.array(val, dtype=np.float32)
